# revision 1
# baseline (speedup 1.0000x reference)
"""CrossTuckerLayer kernel for 8x Trainium2 NeuronCores (Bass/Tile).

Computes y = einsum('bnvade,ABCDEF,oA,pB,qC,aD,dE,eF->bnvopq', ...)
reshaped to [b, n, v, o*p, q], data-parallel over the 2048 (b,n,v) samples
(256 per core). All HBM I/O is bf16 (harness gate is rel_err < 2e-2; this
path lands ~3.4e-3), halving DMA traffic vs fp32.

Host folds the tiny Tucker factors (all <10K params) into two matrices:
  M    [16384, 8] = einsum('ABCDEF,aD,dE,eF->adeABC', core, a0, a1, a2)
  Wout [8, 32768] = einsum('oA,pB,qC->ABCopq', u0, u1, u2)

Per core the 256 samples split into two 128-sample windows:
  stage A (PE): s2_w[8, 128] = sum over 128 fin-chunks of
      M_ck[128f, 8]^T @ x_ck[128f, 128s]; M is the stationary operand so
      the result lands directly in the [8, s] layout stage C needs.
      Both windows run back-to-back, chasing the x DMA stream (the PE is
      nowhere near the bottleneck, so A costs no extra wall-clock).
  s2 is then replicated to partition blocks 0/32/64/96 (one DVE copy +
      three SBUF->SBUF DMAs) so stage C can row-tile the PE.
  stage C (PE): y[128s, 512] tiles = s2_w[8, 128]^T @ Wout[8, 512] with
      K=8 only — so four matmuls run CONCURRENTLY in distinct 32-row
      PE groups via tile_position=(32i, 0) (Wout is staged per row-group
      host-side). Stage C runs uninterrupted — no stage-A matmuls in the
      PE FIFO ahead of C rounds — so its cadence is set purely by the
      psum->sbuf copies (vector/scalar alternating, the only two engines
      that can read PSUM on TRN2) and the y DMA rate. Half-stage y
      stores alternate between the two HWDGE queues so neither queue
      idles across a stage boundary.

DMA issue is staggered via tile-pool reuse (xp bufs): the DMA engines
round-robin across all outstanding transfers on a queue, so issuing
everything upfront makes the FIRST tile complete last.
"""

import numpy as np
import ml_dtypes

import concourse.bass as bass
import concourse.bacc as bacc
import concourse.mybir as mybir
from concourse.tile import TileContext
from concourse.bass_utils import run_bass_kernel_spmd

F32 = mybir.dt.float32
BF16 = mybir.dt.bfloat16
BF = ml_dtypes.bfloat16

NCORES = 8
S_TOT = 2048          # 4*64*8 samples
S = S_TOT // NCORES   # 256 per core
FIN = 16 * 16 * 64    # 16384
FOUT = 256 * 128      # 32768
NCK = FIN // 128      # 128 contraction chunks of 128
WIN = 128             # samples per window
N_WIN = S // WIN      # 2
G_CK = 32             # chunks per x DMA tile: 1MB transfers with 8KB
                      # contiguous per partition read ~355GB/s vs ~285
                      # for 0.5MB tiles (DRAM read locality)
N_G = NCK // G_CK     # 4 x tiles per window
YCHUNK = 512          # one matmul's psum cols (fits a 2KB fp32 bank)
YSTAGE = 4096         # cols per y staging tile / output DMA
N_YSTAGE = FOUT // YSTAGE  # 8 per window
NTILE = 4             # concurrent row-group matmuls in stage C
NSLOT = FOUT // YCHUNK // NTILE  # 16 column slots per row-group


def _host_weights(core, u0, u1, u2, a0, a1, a2):
    """Fold the Tucker factors into M [128f, 128ck*8] and the row-group
    staged Wout [128, NSLOT*512]."""
    M = np.einsum(
        "ABCDEF,aD,dE,eF->adeABC",
        core.astype(np.float64), a0.astype(np.float64),
        a1.astype(np.float64), a2.astype(np.float64),
    ).reshape(FIN, 8)
    # SBUF layout [f, ck*8 + r] where fin = ck*128 + f
    Mdev = np.ascontiguousarray(
        M.reshape(NCK, 128, 8).transpose(1, 0, 2).reshape(128, NCK * 8)
    ).astype(BF)

    Wout = np.einsum(
        "oA,pB,qC->ABCopq",
        u0.astype(np.float64), u1.astype(np.float64), u2.astype(np.float64),
    ).reshape(8, FOUT)
    # chunk c of 512 cols -> row-group i = c % 4, col slot j = c // 4;
    # staged at SBUF partitions 32i..32i+8 (rows 8..31 of each group are
    # padding, never read).
    wl4 = np.zeros((128, NSLOT * YCHUNK), dtype=np.float64)
    for c in range(FOUT // YCHUNK):
        i, j = c % NTILE, c // NTILE
        wl4[32 * i:32 * i + 8, j * YCHUNK:(j + 1) * YCHUNK] = \
            Wout[:, c * YCHUNK:(c + 1) * YCHUNK]
    return Mdev, np.ascontiguousarray(wl4.astype(BF))


def _host_x(x):
    """x [2048, FIN] f32 -> per-core dev layout [128f, w*16K + ck*128 + s]."""
    xb = x.reshape(S_TOT, FIN).astype(BF)
    xd = np.ascontiguousarray(
        xb.reshape(NCORES, N_WIN, WIN, NCK, 128).transpose(0, 4, 1, 3, 2)
    ).reshape(NCORES, 128, N_WIN * FIN)
    return xd


def _build():
    nc = bacc.Bacc("TRN2", target_bir_lowering=False, debug=False)
    x_d = nc.dram_tensor("x", [128, N_WIN * FIN], BF16, kind="ExternalInput")
    m_d = nc.dram_tensor("m", [128, NCK * 8], BF16, kind="ExternalInput")
    wl_d = nc.dram_tensor("wl", [128, NSLOT * YCHUNK], BF16,
                          kind="ExternalInput")
    y_d = nc.dram_tensor("y", [S, FOUT], BF16, kind="ExternalOutput")

    with TileContext(nc) as tc:
        with (
            tc.tile_pool(name="consts", bufs=1) as cpool,
            tc.tile_pool(name="xp", bufs=4) as xp,
            tc.tile_pool(name="s2p", bufs=2) as s2p,
            tc.tile_pool(name="yp", bufs=6) as yp,
            tc.tile_pool(name="psA", bufs=1, space=bass.MemorySpace.PSUM) as psA,
            tc.tile_pool(name="psC", bufs=7, space=bass.MemorySpace.PSUM) as psC,
        ):
            mm = cpool.tile([128, NCK * 8], BF16)
            nc.sync.dma_start(mm[:], m_d[:])
            wl = cpool.tile([128, NSLOT * YCHUNK], BF16)
            nc.scalar.dma_start(wl[:], wl_d[:])

            # x DMAs: issue order == consumption order; xp bufs throttle
            # issue so in-flight transfers stay few (round-robin engines
            # otherwise finish everything at once, starving stage A).
            x_tiles = {}
            for w in range(N_WIN):
                for g in range(N_G):
                    i = w * N_G + g
                    xg = xp.tile([128, G_CK * WIN], BF16, tag="xg",
                                 name=f"x_{w}_{g}")
                    eng = nc.sync if i % 2 == 0 else nc.scalar
                    eng.dma_start(
                        xg[:],
                        x_d[:, (w * NCK + g * G_CK) * WIN:
                               (w * NCK + (g + 1) * G_CK) * WIN],
                    )
                    x_tiles[(w, g)] = xg

            sA = [psA.tile([8, WIN], F32, tag="sA", name=f"sA_{w}")
                  for w in range(N_WIN)]
            s2r = [s2p.tile([128, WIN], BF16, tag="s2", name=f"s2_{w}")
                   for w in range(N_WIN)]

            def emit_a_group(w, g):
                for ckl in range(G_CK):
                    ck = g * G_CK + ckl
                    nc.tensor.matmul(
                        sA[w][:],
                        mm[:, ck * 8:(ck + 1) * 8],
                        x_tiles[(w, g)][:, ckl * WIN:(ckl + 1) * WIN],
                        start=(ck == 0), stop=(ck == NCK - 1),
                        skip_group_check=True,
                    )

            def emit_s2_replicate(w):
                # bf16 downcast into row-group 0, then fan out to 32/64/96
                nc.vector.tensor_copy(s2r[w][0:8, :], sA[w][:])
                for i in range(1, NTILE):
                    nc.sync.dma_start(s2r[w][32 * i:32 * i + 8, :],
                                      s2r[w][0:8, :])

            def emit_c_stage(w, st):
                y_sb = yp.tile([128, YSTAGE], BF16, tag="ysb", name="y_sb")
                for h in range(2):
                    slot = st * 2 + h
                    # 4 concurrent row-group matmuls, one psum bank each
                    pss = []
                    for i in range(NTILE):
                        y_ps = psC.tile([128, YCHUNK], F32, tag="yps",
                                        name="y_ps")
                        nc.tensor.matmul(
                            y_ps[:],
                            s2r[w][32 * i:32 * i + 8, :],
                            wl[32 * i:32 * i + 8,
                               slot * YCHUNK:(slot + 1) * YCHUNK],
                            start=True, stop=True,
                            tile_position=(32 * i, 0),
                        )
                        pss.append(y_ps)
                    # one engine owns this half-stage: the two engines
                    # run concurrent halves, and the scalar engine issues
                    # its own half's store with no cross-engine sem hop
                    for i in range(NTILE):
                        c8 = h * NTILE + i
                        dst = y_sb[:, c8 * YCHUNK:(c8 + 1) * YCHUNK]
                        if h == 0:
                            nc.vector.tensor_copy(dst, pss[i][:])
                        else:
                            nc.scalar.copy(dst, pss[i][:])
                    eng = nc.sync if h == 0 else nc.scalar
                    eng.dma_start(
                        y_d[w * WIN:(w + 1) * WIN,
                            st * YSTAGE + h * 4 * YCHUNK:
                            st * YSTAGE + (h + 1) * 4 * YCHUNK],
                        y_sb[:, h * 4 * YCHUNK:(h + 1) * 4 * YCHUNK],
                    )

            # stage A for both windows, chasing the x DMA stream; then
            # stage C uninterrupted so copies/y-DMA pace the pipeline
            # with no stage-A matmuls in the PE FIFO ahead of C rounds.
            for w in range(N_WIN):
                for g in range(N_G):
                    emit_a_group(w, g)
                emit_s2_replicate(w)
            for w in range(N_WIN):
                for st in range(N_YSTAGE):
                    emit_c_stage(w, st)
    nc.compile()
    return nc


_NC_CACHE = []


def _get_nc():
    if not _NC_CACHE:
        _NC_CACHE.append(_build())
    return _NC_CACHE[0]


def run(inputs, trace=False):
    x = np.asarray(inputs["x"], dtype=np.float32)
    Mdev, wl4 = _host_weights(
        np.asarray(inputs["core"]),
        np.asarray(inputs["u0"]), np.asarray(inputs["u1"]),
        np.asarray(inputs["u2"]),
        np.asarray(inputs["a0"]), np.asarray(inputs["a1"]),
        np.asarray(inputs["a2"]),
    )
    xd = _host_x(x)
    nc = _get_nc()
    in_maps = []
    for i in range(NCORES):
        in_maps.append({
            "x": xd[i],
            "m": Mdev,
            "wl": wl4,
        })
    res = run_bass_kernel_spmd(
        nc, in_maps, core_ids=list(range(NCORES)), trace=trace,
    )
    y = np.concatenate([np.asarray(r["y"]) for r in res.results], axis=0)
    y = y.astype(np.float32).reshape(4, 64, 8, 256, 128)
    return y, res


def kernel(**inputs) -> np.ndarray:
    y, _ = run(inputs, trace=False)
    return y



# revision 2
# speedup vs baseline: 1.0155x; 1.0155x over previous
"""CrossTuckerLayer kernel for 8x Trainium2 NeuronCores (Bass/Tile).

Computes y = einsum('bnvade,ABCDEF,oA,pB,qC,aD,dE,eF->bnvopq', ...)
reshaped to [b, n, v, o*p, q], data-parallel over the 2048 (b,n,v) samples
(256 per core). All HBM I/O is bf16 (harness gate is rel_err < 2e-2; this
path lands ~3.4e-3), halving DMA traffic vs fp32.

Host folds the tiny Tucker factors (all <10K params) into two matrices:
  M    [16384, 8] = einsum('ABCDEF,aD,dE,eF->adeABC', core, a0, a1, a2)
  Wout [8, 32768] = einsum('oA,pB,qC->ABCopq', u0, u1, u2)

Per core the 256 samples split into two 128-sample windows:
  stage A (PE): s2_w[8, 128] = sum over 128 fin-chunks of
      M_ck[128f, 8]^T @ x_ck[128f, 128s].
  s2 is then replicated to partition blocks 0/32/64/96 (one DVE copy +
      three SBUF->SBUF DMAs) so stage C can row-tile the PE.
  stage C (PE): y[128s, 512] tiles = s2_w[8, 128]^T @ W[8, 512] with
      K=8 only — four matmuls run CONCURRENTLY in distinct 32-row
      PE groups via tile_position=(32i, 0). Wout is shipped as a
      0.5MB group-permuted [8, 4*8192] tensor and DMA'd straight into
      the four 32-row SBUF slots (no 4x redundant HBM traffic).

Schedule (the whole problem is HBM-bound: ~8MB x read + 16MB y write +
0.75MB weights per core at ~330GB/s => ~75us of DMA):
  - x w0 lands via a small first tile (8 chunks) + staggered 1MB tiles;
    weight DMAs are tiny and share the two HWDGE queues.
  - x w1 tile issue is deferred via tile-pool rings (bufs) so the early
    round-robin DMA service goes to w0 (first A matmul ~6us, not 22us).
  - stage C(w0) column-stages are interleaved with stage A(w1) chunk
    groups in the PE stream, so y w0 writes overlap x w1 reads and the
    DMA queues never drain between phases.
  - y staging tiles are 8192 cols (16KB bf16/partition); each half-DMA
    writes 4096 cols = 8KB contiguous per row for good HBM locality.
"""

import numpy as np
import ml_dtypes

import concourse.bass as bass
import concourse.bacc as bacc
import concourse.mybir as mybir
from concourse.tile import TileContext
from concourse.bass_utils import run_bass_kernel_spmd

F32 = mybir.dt.float32
BF16 = mybir.dt.bfloat16
BF = ml_dtypes.bfloat16

NCORES = 8
S_TOT = 2048          # 4*64*8 samples
S = S_TOT // NCORES   # 256 per core
FIN = 16 * 16 * 64    # 16384
FOUT = 256 * 128      # 32768
NCK = FIN // 128      # 128 contraction chunks of 128
WIN = 128             # samples per window
N_WIN = S // WIN      # 2
YCHUNK = 512          # one matmul's psum cols (fits a 2KB fp32 bank)
YSTAGE = 8192         # cols per y staging tile (two 4096-col DMA halves)
N_YSTAGE = FOUT // YSTAGE  # 4 per window
NTILE = 4             # concurrent row-group matmuls in stage C
NSLOT = FOUT // YCHUNK // NTILE  # 16 column slots per row-group

# x tile chunk-groups per window: (window, start_chunk, n_chunks)
W0_GROUPS = [(0, 0, 8), (0, 8, 24), (0, 32, 32), (0, 64, 32), (0, 96, 32)]
W1_GROUPS = [(1, 0, 32), (1, 32, 32), (1, 64, 32), (1, 96, 32)]


def _host_weights(core, u0, u1, u2, a0, a1, a2):
    """Fold the Tucker factors into M [128f, 128ck*8] and the
    group-permuted Wout wl_g [8, 4*NSLOT*512]."""
    M = np.einsum(
        "ABCDEF,aD,dE,eF->adeABC",
        core.astype(np.float64), a0.astype(np.float64),
        a1.astype(np.float64), a2.astype(np.float64),
    ).reshape(FIN, 8)
    # SBUF layout [f, ck*8 + r] where fin = ck*128 + f
    Mdev = np.ascontiguousarray(
        M.reshape(NCK, 128, 8).transpose(1, 0, 2).reshape(128, NCK * 8)
    ).astype(BF)

    Wout = np.einsum(
        "oA,pB,qC->ABCopq",
        u0.astype(np.float64), u1.astype(np.float64), u2.astype(np.float64),
    ).reshape(8, FOUT)
    # chunk c of 512 cols -> row-group i = c % 4, col slot j = c // 4;
    # wl_g packs each group's 16 slots contiguously so the device can DMA
    # group i straight into SBUF partitions 32i..32i+8.
    wl_g = np.zeros((8, NTILE * NSLOT * YCHUNK), dtype=np.float64)
    for c in range(FOUT // YCHUNK):
        i, j = c % NTILE, c // NTILE
        wl_g[:, (i * NSLOT + j) * YCHUNK:(i * NSLOT + j + 1) * YCHUNK] = \
            Wout[:, c * YCHUNK:(c + 1) * YCHUNK]
    return Mdev, np.ascontiguousarray(wl_g.astype(BF))


def _host_x(x):
    """x [2048, FIN] f32 -> per-core dev layout [128f, w*16K + ck*128 + s]."""
    xb = x.reshape(S_TOT, FIN).astype(BF)
    xd = np.ascontiguousarray(
        xb.reshape(NCORES, N_WIN, WIN, NCK, 128).transpose(0, 4, 1, 3, 2)
    ).reshape(NCORES, 128, N_WIN * FIN)
    return xd


def _build():
    nc = bacc.Bacc("TRN2", target_bir_lowering=False, debug=False)
    x_d = nc.dram_tensor("x", [128, N_WIN * FIN], BF16, kind="ExternalInput")
    m_d = nc.dram_tensor("m", [128, NCK * 8], BF16, kind="ExternalInput")
    wl_d = nc.dram_tensor("wl", [8, NTILE * NSLOT * YCHUNK], BF16,
                          kind="ExternalInput")
    y_d = nc.dram_tensor("y", [S, FOUT], BF16, kind="ExternalOutput")

    with TileContext(nc) as tc:
        with (
            tc.tile_pool(name="consts", bufs=1) as cpool,
            tc.tile_pool(name="xs0", bufs=1) as xs0,
            tc.tile_pool(name="xs1", bufs=1) as xs1,
            tc.tile_pool(name="xA", bufs=2) as xA,
            tc.tile_pool(name="xB", bufs=1) as xB,
            tc.tile_pool(name="s2p", bufs=2) as s2p,
            tc.tile_pool(name="yp", bufs=4) as yp,
            tc.tile_pool(name="psA", bufs=1, space=bass.MemorySpace.PSUM) as psA,
            tc.tile_pool(name="psC", bufs=7, space=bass.MemorySpace.PSUM) as psC,
        ):
            # --- DMA issue schedule -------------------------------------
            # sync:   mm, x(w0,t0 small), x(w0,t2), x(w0,t4), wl g0, wl g2,
            #         then ring-deferred x(w1,t1), x(w1,t3), then y h0s.
            # scalar: x(w0,t1), x(w0,t3), wl g1, wl g3, ring-deferred
            #         x(w1,t0), x(w1,t2), then y h1s.
            mm = cpool.tile([128, NCK * 8], BF16)
            nc.sync.dma_start(mm[:], m_d[:])

            x_tiles = {}

            def issue_x(eng, pool, tag, w, ck0, n):
                xg = pool.tile([128, n * WIN], BF16, tag=tag,
                               name=f"x_{w}_{ck0}")
                eng.dma_start(
                    xg[:],
                    x_d[:, (w * NCK + ck0) * WIN:(w * NCK + ck0 + n) * WIN],
                )
                x_tiles[(w, ck0)] = xg

            issue_x(nc.sync, xs0, "x0", 0, 0, 8)       # 0.25MB, lands first
            issue_x(nc.scalar, xs1, "x1", 0, 8, 24)    # 0.75MB
            issue_x(nc.sync, xA, "xa", 0, 32, 32)      # 1MB
            issue_x(nc.scalar, xB, "xb", 0, 64, 32)
            issue_x(nc.sync, xA, "xa", 0, 96, 32)

            wl = cpool.tile([128, NSLOT * YCHUNK], BF16)
            for i in range(NTILE):
                eng = nc.sync if i % 2 == 0 else nc.scalar
                eng.dma_start(
                    wl[32 * i:32 * i + 8, :],
                    wl_d[:, i * NSLOT * YCHUNK:(i + 1) * NSLOT * YCHUNK],
                )

            # w1 tiles: xA ring (bufs=2) defers issue until the matching w0
            # tile is consumed; xB ring (bufs=1) likewise.
            issue_x(nc.scalar, xB, "xb", 1, 0, 32)   # waits x(0,64) consumed
            issue_x(nc.sync, xA, "xa", 1, 32, 32)    # waits x(0,32) consumed
            issue_x(nc.scalar, xB, "xb", 1, 64, 32)  # waits x(1,0) consumed
            issue_x(nc.sync, xA, "xa", 1, 96, 32)    # waits x(0,96) consumed

            sA = [psA.tile([8, WIN], F32, tag="sA", name=f"sA_{w}")
                  for w in range(N_WIN)]
            s2r = [s2p.tile([128, WIN], BF16, tag="s2", name=f"s2_{w}")
                   for w in range(N_WIN)]

            def emit_a_group(w, ck0, n):
                xg = x_tiles[(w, ck0)]
                for l in range(n):
                    ck = ck0 + l
                    nc.tensor.matmul(
                        sA[w][:],
                        mm[:, ck * 8:(ck + 1) * 8],
                        xg[:, l * WIN:(l + 1) * WIN],
                        start=(ck == 0), stop=(ck == NCK - 1),
                        skip_group_check=True,
                    )

            def emit_s2_replicate(w):
                # bf16 downcast into row-group 0, then fan out to 32/64/96
                nc.vector.tensor_copy(s2r[w][0:8, :], sA[w][:])
                nc.sync.dma_start(s2r[w][32:40, :], s2r[w][0:8, :])
                nc.scalar.dma_start(s2r[w][64:72, :], s2r[w][0:8, :])
                nc.sync.dma_start(s2r[w][96:104, :], s2r[w][0:8, :])

            def emit_c_stage(w, st):
                y_sb = yp.tile([128, YSTAGE], BF16, tag="ysb", name="y_sb")
                for jl in range(4):
                    j = st * 4 + jl
                    for i in range(NTILE):
                        y_ps = psC.tile([128, YCHUNK], F32, tag="yps",
                                        name="y_ps")
                        nc.tensor.matmul(
                            y_ps[:],
                            s2r[w][32 * i:32 * i + 8, :],
                            wl[32 * i:32 * i + 8,
                               j * YCHUNK:(j + 1) * YCHUNK],
                            start=True, stop=True,
                            tile_position=(32 * i, 0),
                        )
                        dst = y_sb[:, (jl * NTILE + i) * YCHUNK:
                                   (jl * NTILE + i + 1) * YCHUNK]
                        # vector owns cols 0..4095, scalar owns 4096..8191
                        if jl < 2:
                            nc.vector.tensor_copy(dst, y_ps[:])
                        else:
                            nc.scalar.copy(dst, y_ps[:])
                half = 4096
                nc.sync.dma_start(
                    y_d[w * WIN:(w + 1) * WIN,
                        st * YSTAGE:st * YSTAGE + half],
                    y_sb[:, 0:half],
                )
                nc.scalar.dma_start(
                    y_d[w * WIN:(w + 1) * WIN,
                        st * YSTAGE + half:(st + 1) * YSTAGE],
                    y_sb[:, half:YSTAGE],
                )

            # stage A w0 chases the x stream; C(w0) stages interleave with
            # A(w1) chunk groups so y writes overlap the w1 x reads.
            for (w, ck0, n) in W0_GROUPS:
                emit_a_group(w, ck0, n)
            emit_s2_replicate(0)
            for st in range(N_YSTAGE):
                emit_c_stage(0, st)
                emit_a_group(*W1_GROUPS[st])
            emit_s2_replicate(1)
            for st in range(N_YSTAGE):
                emit_c_stage(1, st)
    nc.compile()
    return nc


_NC_CACHE = []


def _get_nc():
    if not _NC_CACHE:
        _NC_CACHE.append(_build())
    return _NC_CACHE[0]


def run(inputs, trace=False):
    x = np.asarray(inputs["x"], dtype=np.float32)
    Mdev, wl_g = _host_weights(
        np.asarray(inputs["core"]),
        np.asarray(inputs["u0"]), np.asarray(inputs["u1"]),
        np.asarray(inputs["u2"]),
        np.asarray(inputs["a0"]), np.asarray(inputs["a1"]),
        np.asarray(inputs["a2"]),
    )
    xd = _host_x(x)
    nc = _get_nc()
    in_maps = []
    for i in range(NCORES):
        in_maps.append({
            "x": xd[i],
            "m": Mdev,
            "wl": wl_g,
        })
    res = run_bass_kernel_spmd(
        nc, in_maps, core_ids=list(range(NCORES)), trace=trace,
    )
    y = np.concatenate([np.asarray(r["y"]) for r in res.results], axis=0)
    y = y.astype(np.float32).reshape(4, 64, 8, 256, 128)
    return y, res


def kernel(**inputs) -> np.ndarray:
    y, _ = run(inputs, trace=False)
    return y


# revision 5
# speedup vs baseline: 1.1083x; 1.0914x over previous
"""CrossTuckerLayer kernel for 8x Trainium2 NeuronCores (Bass/Tile).

Computes y = einsum('bnvade,ABCDEF,oA,pB,qC,aD,dE,eF->bnvopq', ...)
reshaped to [b, n, v, o*p, q], data-parallel over the 2048 (b,n,v) samples
(256 per core). All HBM I/O is bf16 (harness gate is rel_err < 2e-2; this
path lands ~3.4e-3), halving DMA traffic vs fp32.

Host folds the tiny Tucker factors (all <10K params) into two matrices:
  M    [16384, 8] = einsum('ABCDEF,aD,dE,eF->adeABC', core, a0, a1, a2)
  Wout [8, 32768] = einsum('oA,pB,qC->ABCopq', u0, u1, u2)

Per core the 256 samples split into two 128-sample windows:
  stage A (PE): s2_w[8, 128] = sum over 128 fin-chunks of
      M_ck[128f, 8]^T @ x_ck[128f, 128s].
  s2 is then replicated to partition blocks 0/32/64/96 (one DVE copy +
      three SBUF->SBUF DMAs) so stage C can row-tile the PE.
  stage C (PE): y[128s, 512] tiles = s2_w[8, 128]^T @ W[8, 512] with
      K=8 only — four matmuls run CONCURRENTLY in distinct 32-row
      PE groups via tile_position=(32i, 0). Wout is shipped as a
      0.5MB group-permuted [8, 4*8192] tensor and DMA'd straight into
      the four 32-row SBUF slots (no 4x redundant HBM traffic).

Schedule (HBM-bound: ~8MB x read + 16MB y write + 0.75MB weights per
core; the two HWDGE queues share 16 DMA engines at ~330GB/s aggregate):
  - ALL x tiles are issued upfront: the DMA engines round-robin packets
    across outstanding descriptors, and queues need several active
    descriptors to sustain full rate. The first x tile is small (8
    chunks) and M is split into chunk-aligned slivers so stage A starts
    ~5us in; later tiles are 1.5-2MB for DRAM read locality.
  - stage A(w1) is emitted in 16-chunk slices between C(w0) stages, so
    the PE finishes s2(w1) while C(w0) copies run and y w0 writes
    overlap the x w1 reads; the DMA queues never drain.
  - y staging tiles are 8192 cols; each half-DMA writes 4096 cols = 8KB
    contiguous per row; halves alternate between the two queues.
"""

import numpy as np
import ml_dtypes

import concourse.bass as bass
import concourse.bacc as bacc
import concourse.mybir as mybir
from concourse.tile import TileContext
from concourse.bass_utils import run_bass_kernel_spmd

F32 = mybir.dt.float32
BF16 = mybir.dt.bfloat16
BF = ml_dtypes.bfloat16

NCORES = 8
S_TOT = 2048          # 4*64*8 samples
S = S_TOT // NCORES   # 256 per core
FIN = 16 * 16 * 64    # 16384
FOUT = 256 * 128      # 32768
NCK = FIN // 128      # 128 contraction chunks of 128
WIN = 128             # samples per window
N_WIN = S // WIN      # 2
YCHUNK = 512          # one matmul's psum cols (fits a 2KB fp32 bank)
YSTAGE = 8192         # cols per y staging tile (two 4096-col DMA halves)
N_YSTAGE = FOUT // YSTAGE  # 4 per window
NTILE = 4             # concurrent row-group matmuls in stage C
NSLOT = FOUT // YCHUNK // NTILE  # 16 column slots per row-group

# chunk-range boundaries shared by the x tiles (w0), mm slivers
MM_SPLITS = [(0, 8), (8, 24), (32, 48), (80, 48)]
W0_TILES = [(0, 0, 8), (0, 8, 24), (0, 32, 48), (0, 80, 48)]
W1_TILES = [(1, 0, 64), (1, 64, 64)]


def _host_weights(core, u0, u1, u2, a0, a1, a2):
    """Fold the Tucker factors into M [128f, 128ck*8] and the
    group-permuted Wout wl_g [8, 4*NSLOT*512]."""
    M = np.einsum(
        "ABCDEF,aD,dE,eF->adeABC",
        core.astype(np.float64), a0.astype(np.float64),
        a1.astype(np.float64), a2.astype(np.float64),
    ).reshape(FIN, 8)
    # SBUF layout [f, ck*8 + r] where fin = ck*128 + f
    Mdev = np.ascontiguousarray(
        M.reshape(NCK, 128, 8).transpose(1, 0, 2).reshape(128, NCK * 8)
    ).astype(BF)

    Wout = np.einsum(
        "oA,pB,qC->ABCopq",
        u0.astype(np.float64), u1.astype(np.float64), u2.astype(np.float64),
    ).reshape(8, FOUT)
    # chunk c of 512 cols -> row-group i = c % 4, col slot j = c // 4;
    # wl_g packs each group's 16 slots contiguously so the device can DMA
    # group i straight into SBUF partitions 32i..32i+8.
    wl_g = np.zeros((8, NTILE * NSLOT * YCHUNK), dtype=np.float64)
    for c in range(FOUT // YCHUNK):
        i, j = c % NTILE, c // NTILE
        wl_g[:, (i * NSLOT + j) * YCHUNK:(i * NSLOT + j + 1) * YCHUNK] = \
            Wout[:, c * YCHUNK:(c + 1) * YCHUNK]
    return Mdev, np.ascontiguousarray(wl_g.astype(BF))


def _host_x(x):
    """x [2048, FIN] f32 -> per-core dev layout [128f, w*16K + ck*128 + s]."""
    xb = x.reshape(S_TOT, FIN).astype(BF)
    xd = np.ascontiguousarray(
        xb.reshape(NCORES, N_WIN, WIN, NCK, 128).transpose(0, 4, 1, 3, 2)
    ).reshape(NCORES, 128, N_WIN * FIN)
    return xd


def _build():
    nc = bacc.Bacc("TRN2", target_bir_lowering=False, debug=False)
    x_d = nc.dram_tensor("x", [128, N_WIN * FIN], BF16, kind="ExternalInput")
    m_d = nc.dram_tensor("m", [128, NCK * 8], BF16, kind="ExternalInput")
    wl_d = nc.dram_tensor("wl", [8, NTILE * NSLOT * YCHUNK], BF16,
                          kind="ExternalInput")
    y_d = nc.dram_tensor("y", [S, FOUT], BF16, kind="ExternalOutput")

    with TileContext(nc) as tc:
        with (
            tc.tile_pool(name="consts", bufs=1) as cpool,
            tc.tile_pool(name="xp", bufs=1) as xp,
            tc.tile_pool(name="s2p", bufs=2) as s2p,
            tc.tile_pool(name="yp", bufs=4) as yp,
            tc.tile_pool(name="psA", bufs=1, space=bass.MemorySpace.PSUM) as psA,
            tc.tile_pool(name="psC", bufs=7, space=bass.MemorySpace.PSUM) as psC,
        ):
            # --- DMA issue schedule: everything upfront ------------------
            # mm slivers are tiny and chunk-aligned so A g0 starts ~5us in.
            mm_tiles = {}
            for (ck0, n) in MM_SPLITS:
                mmt = cpool.tile([128, n * 8], BF16, name=f"mm_{ck0}")
                nc.sync.dma_start(mmt[:], m_d[:, ck0 * 8:(ck0 + n) * 8])
                mm_tiles[(ck0, n)] = mmt

            def mm_for(ck):
                for (ck0, n) in MM_SPLITS:
                    if ck0 <= ck < ck0 + n:
                        t = mm_tiles[(ck0, n)]
                        return t[:, (ck - ck0) * 8:(ck - ck0 + 1) * 8]
                raise AssertionError(ck)

            x_tiles = {}

            def issue_x(eng, w, ck0, n):
                xg = xp.tile([128, n * WIN], BF16, tag=f"x_{w}_{ck0}",
                             name=f"x_{w}_{ck0}")
                eng.dma_start(
                    xg[:],
                    x_d[:, (w * NCK + ck0) * WIN:(w * NCK + ck0 + n) * WIN],
                )
                x_tiles[(w, ck0)] = xg

            def x_for(w, ck):
                for (ww, ck0, n) in (W0_TILES + W1_TILES):
                    if ww == w and ck0 <= ck < ck0 + n:
                        xg = x_tiles[(w, ck0)]
                        return xg[:, (ck - ck0) * WIN:(ck - ck0 + 1) * WIN]
                raise AssertionError((w, ck))

            issue_x(nc.sync, 0, 0, 8)       # 0.25MB, lands first
            issue_x(nc.scalar, 0, 8, 24)    # 0.75MB
            issue_x(nc.sync, 0, 32, 48)     # 1.5MB
            issue_x(nc.scalar, 0, 80, 48)   # 1.5MB
            issue_x(nc.sync, 1, 0, 64)      # 2MB
            issue_x(nc.scalar, 1, 64, 64)   # 2MB

            wl = cpool.tile([128, NSLOT * YCHUNK], BF16)
            for i in range(NTILE):
                eng = nc.sync if i % 2 == 0 else nc.scalar
                eng.dma_start(
                    wl[32 * i:32 * i + 8, :],
                    wl_d[:, i * NSLOT * YCHUNK:(i + 1) * NSLOT * YCHUNK],
                )

            sA = [psA.tile([8, WIN], F32, tag="sA", name=f"sA_{w}")
                  for w in range(N_WIN)]
            s2r = [s2p.tile([128, WIN], BF16, tag="s2", name=f"s2_{w}")
                   for w in range(N_WIN)]

            def emit_a_slice(w, ck0, n):
                for ck in range(ck0, ck0 + n):
                    nc.tensor.matmul(
                        sA[w][:],
                        mm_for(ck),
                        x_for(w, ck),
                        start=(ck == 0), stop=(ck == NCK - 1),
                        skip_group_check=True,
                    )

            def emit_s2_replicate(w):
                # bf16 downcast into row-group 0, then fan out to 32/64/96
                nc.vector.tensor_copy(s2r[w][0:8, :], sA[w][:])
                nc.sync.dma_start(s2r[w][32:40, :], s2r[w][0:8, :])
                nc.sync.dma_start(s2r[w][64:72, :], s2r[w][0:8, :])
                nc.sync.dma_start(s2r[w][96:104, :], s2r[w][0:8, :])

            def emit_c_stage(w, st):
                y_sb = yp.tile([128, YSTAGE], BF16, tag="ysb", name="y_sb")
                for jl in range(4):
                    j = st * 4 + jl
                    for i in range(NTILE):
                        y_ps = psC.tile([128, YCHUNK], F32, tag="yps",
                                        name="y_ps")
                        nc.tensor.matmul(
                            y_ps[:],
                            s2r[w][32 * i:32 * i + 8, :],
                            wl[32 * i:32 * i + 8,
                               j * YCHUNK:(j + 1) * YCHUNK],
                            start=True, stop=True,
                            tile_position=(32 * i, 0),
                        )
                        dst = y_sb[:, (jl * NTILE + i) * YCHUNK:
                                   (jl * NTILE + i + 1) * YCHUNK]
                        # vector owns cols 0..4095, scalar owns 4096..8191
                        if jl < 2:
                            nc.vector.tensor_copy(dst, y_ps[:])
                        else:
                            nc.scalar.copy(dst, y_ps[:])
                half = 4096
                nc.sync.dma_start(
                    y_d[w * WIN:(w + 1) * WIN,
                        st * YSTAGE:st * YSTAGE + half],
                    y_sb[:, 0:half],
                )
                nc.scalar.dma_start(
                    y_d[w * WIN:(w + 1) * WIN,
                        st * YSTAGE + half:(st + 1) * YSTAGE],
                    y_sb[:, half:YSTAGE],
                )

            # stage A w0 chases the x stream; A(w1) 16-chunk slices are
            # spread between C(w0) stages (PE runs ahead of the copies, so
            # the slices hide inside the copy-paced stage cadence).
            for (w, ck0, n) in W0_TILES:
                emit_a_slice(w, ck0, n)
            emit_s2_replicate(0)
            emit_c_stage(0, 0)
            emit_c_stage(0, 1)
            emit_a_slice(1, 0, 16)
            emit_a_slice(1, 16, 16)
            emit_c_stage(0, 2)
            emit_a_slice(1, 32, 16)
            emit_a_slice(1, 48, 16)
            emit_c_stage(0, 3)
            for k in range(64, 128, 16):
                emit_a_slice(1, k, 16)
            emit_s2_replicate(1)
            for st in range(N_YSTAGE):
                emit_c_stage(1, st)
    nc.compile()
    return nc


_NC_CACHE = []


def _get_nc():
    if not _NC_CACHE:
        _NC_CACHE.append(_build())
    return _NC_CACHE[0]


def run(inputs, trace=False):
    x = np.asarray(inputs["x"], dtype=np.float32)
    Mdev, wl_g = _host_weights(
        np.asarray(inputs["core"]),
        np.asarray(inputs["u0"]), np.asarray(inputs["u1"]),
        np.asarray(inputs["u2"]),
        np.asarray(inputs["a0"]), np.asarray(inputs["a1"]),
        np.asarray(inputs["a2"]),
    )
    xd = _host_x(x)
    nc = _get_nc()
    in_maps = []
    for i in range(NCORES):
        in_maps.append({
            "x": xd[i],
            "m": Mdev,
            "wl": wl_g,
        })
    res = run_bass_kernel_spmd(
        nc, in_maps, core_ids=list(range(NCORES)), trace=trace,
    )
    y = np.concatenate([np.asarray(r["y"]) for r in res.results], axis=0)
    y = y.astype(np.float32).reshape(4, 64, 8, 256, 128)
    return y, res


def kernel(**inputs) -> np.ndarray:
    y, _ = run(inputs, trace=False)
    return y
